# revision 8
# baseline (speedup 1.0000x reference)
"""Trainium2 Bass kernel for nn_EnsembleLayer (MoE one-hot routing).

Reference math (per token i, expert e = expert_idx[i]):
    out[i] = W[e] @ logits[i] + b[e] + W_prior[e] @ prior_logits[i] + b_prior[e]

Strategy:
  * Host-side routing: sort tokens by expert (the reference wastefully
    computes all 16 experts per token; we compute only the routed one).
  * Expert-parallel over 8 NeuronCores: core c owns experts (2c, 2c+1),
    each padded to a static capacity of C tokens.
  * Per expert slot the device computes  V.T @ Z  where
        Z = [X_e.T ; Xp_e.T]          (K=2048, C tokens)
        V = [W[e].T ; W_prior[e].T]   (K=2048, 1024 out)
    i.e. posterior and prior fused into one K=2048 contraction, both
    accumulating in the same PSUM tile.
  * Bias (zero in this problem, but handled anyway) is added on host.
  * Tokens overflowing the static capacity (cannot happen for the fixed
    seed's counts, max 295 < C) fall back to exact numpy on host.

Precision strategy for the matmuls (PE TensorEngine):
  * "fp32r" : single-pass FP32R (1s+8e+11m, 1 cyc/row at N>=256). ~1.5e-4
              max scale-relative error.
  * "bf16x3": bf16 hi/lo split, 3 accumulating passes
              (hi@hi + hi@lo + lo@hi). ~5e-6 max scale-relative error.
  * "fp32"  : native fp32 matmul (4 cyc/row). ~2e-7.
"""

import sys

sys.path.insert(0, "/opt/trn_rl_repo")

import ml_dtypes
import numpy as np

import concourse.mybir as mybir
import concourse.tile as tile
from concourse import bacc
from concourse.bass_utils import run_bass_kernel_spmd

dt = mybir.dt

# ---- problem constants (hardcoded per contract) ----
B = 4096
IN_F = 1024
OUT_F = 1024
E = 16
N_CORES = 8
EXPERTS_PER_CORE = E // N_CORES  # 2
P = 128
K = 2 * IN_F  # 2048: posterior + prior fused contraction
KO = K // P  # 16 k-tiles
MO = OUT_F // P  # 8 m-tiles
C = 320  # static token capacity per expert (seed-0 max count is 295)

STRATEGY = "bf16x3"  # one of: "fp32r", "bf16x3", "fp32"


def _round_fp32r(x: np.ndarray) -> np.ndarray:
    """Round fp32 to the FP32R grid (1s+8e+11m, RN-even)."""
    u = np.ascontiguousarray(x).view(np.uint32)
    r = (u + np.uint32(0x7FF) + ((u >> np.uint32(12)) & np.uint32(1))) & np.uint32(
        0xFFFFF000
    )
    return r.view(np.float32)


def _split_bf16(x: np.ndarray):
    hi = x.astype(ml_dtypes.bfloat16)
    lo = (x - hi.astype(np.float32)).astype(ml_dtypes.bfloat16)
    return hi, lo


def build_nc(strategy: str = STRATEGY):
    """Build the SPMD per-core Bass program.

    Inputs per core (leading dim j = expert slot 0/1):
      zt  [2, P, KO, C]      tokens, k-tiled transposed   (fp32r / fp32)
      vt  [2, MO, P, KO, P]  weights, k&m-tiled           (fp32r / fp32)
      (bf16x3: zhi/zlo and vhi/vlo in bf16 instead)
    Output:
      yt  [2, MO, P, C] fp32   yt[j,m,mi,n] = out-feature m*P+mi of token n
    """
    nc = bacc.Bacc("TRN2", target_bir_lowering=False, debug=False)

    if strategy == "bf16x3":
        zhi = nc.dram_tensor("zhi", [2, P, KO, C], dt.bfloat16, kind="ExternalInput").ap()
        zlo = nc.dram_tensor("zlo", [2, P, KO, C], dt.bfloat16, kind="ExternalInput").ap()
        vhi = nc.dram_tensor("vhi", [2, MO, P, KO, P], dt.bfloat16, kind="ExternalInput").ap()
        vlo = nc.dram_tensor("vlo", [2, MO, P, KO, P], dt.bfloat16, kind="ExternalInput").ap()
    else:
        mm_dt = dt.float32r if strategy == "fp32r" else dt.float32
        zt = nc.dram_tensor("zt", [2, P, KO, C], mm_dt, kind="ExternalInput").ap()
        vt = nc.dram_tensor("vt", [2, MO, P, KO, P], mm_dt, kind="ExternalInput").ap()
    yt = nc.dram_tensor("yt", [2, MO, P, C], dt.float32, kind="ExternalOutput").ap()

    with tile.TileContext(nc) as tc:
        with (
            tc.tile_pool(name="zp", bufs=1) as zp,
            tc.tile_pool(name="wp", bufs=4) as wp,
            tc.tile_pool(name="ps", bufs=8, space="PSUM") as ps,
            tc.tile_pool(name="op", bufs=4) as op,
        ):
            # Token tiles are chunked along k (KCH k-tiles per chunk) so the
            # first matmuls only wait on the first small DMA instead of the
            # full 5MB token load — PE starts ~17us earlier.
            KCH = 2
            NCH = KO // KCH

            def load_chunked(dram_ap, dtype, tag, nch, sub):
                """Allocate nch tiles [P, KO//nch, C-like] and DMA each."""
                tiles = []
                for ci in range(nch):
                    t = zp.tile([P, KO // nch, dram_ap.shape[-1]], dtype,
                                tag=f"{tag}c{ci}")
                    tiles.append(t)
                # DMAs issued by caller in the order it wants
                return tiles

            def issue(tiles, dram_ap, nch):
                kw = KO // nch
                for ci, t in enumerate(tiles):
                    nc.sync.dma_start(t[:], dram_ap[:, ci * kw : (ci + 1) * kw])

            # Issue order shapes delivery: group-0 weights + j=0 tokens come
            # first; j=1 tokens (not needed until group 8) are drip-fed from
            # inside the group loop so groups 1..7's weights aren't queued
            # behind them.
            kw = KO // NCH

            def issue_chunk(tiles, dram_ap, ci):
                t = tiles[ci]
                nc.sync.dma_start(t[:], dram_ap[:, ci * kw : (ci + 1) * kw])

            pending = []
            if strategy == "bf16x3":
                z_hi = [load_chunked(zhi[j], dt.bfloat16, f"zhi{j}", NCH, C) for j in range(2)]
                z_lo = [load_chunked(zlo[j], dt.bfloat16, f"zlo{j}", NCH, C) for j in range(2)]
                wh0 = wp.tile([P, KO, P], dt.bfloat16, tag="wh")
                wl0 = wp.tile([P, KO, P], dt.bfloat16, tag="wl")
                nc.sync.dma_start(wh0[:, : KO // 2], vhi[0, 0, :, : KO // 2])
                issue_chunk(z_hi[0], zhi[0], 0)
                nc.sync.dma_start(wh0[:, KO // 2 :], vhi[0, 0, :, KO // 2 :])
                for ci in range(1, NCH):
                    issue_chunk(z_hi[0], zhi[0], ci)
                issue_chunk(z_lo[0], zlo[0], 0)
                nc.sync.dma_start(wl0[:, : KO // 2], vlo[0, 0, :, : KO // 2])
                for ci in range(1, NCH):
                    issue_chunk(z_lo[0], zlo[0], ci)
                nc.sync.dma_start(wl0[:, KO // 2 :], vlo[0, 0, :, KO // 2 :])
                for ci in range(NCH):
                    pending.append((z_hi[1], zhi[1], ci))
                    pending.append((z_lo[1], zlo[1], ci))
            else:
                z_tiles = [load_chunked(zt[j], zt.dtype, f"z{j}", NCH, C) for j in range(2)]
                w0 = wp.tile([P, KO, P], vt.dtype, tag="w")
                nc.sync.dma_start(w0[:, : KO // 2], vt[0, 0, :, : KO // 2])
                issue_chunk(z_tiles[0], zt[0], 0)
                nc.sync.dma_start(w0[:, KO // 2 :], vt[0, 0, :, KO // 2 :])
                for ci in range(1, NCH):
                    issue_chunk(z_tiles[0], zt[0], ci)
                for ci in range(NCH):
                    pending.append((z_tiles[1], zt[1], ci))

            for j in range(2):
                for m in range(MO):
                    first = j == 0 and m == 0
                    # drip-feed the j=1 token chunks during early j=0 groups
                    if j == 0 and 3 <= m <= 6 and pending:
                        for _ in range(4):
                            if pending:
                                tiles_, ap_, ci_ = pending.pop(0)
                                issue_chunk(tiles_, ap_, ci_)
                    if strategy == "bf16x3":
                        if first:
                            wh, wl = wh0, wl0
                        else:
                            wh = wp.tile([P, KO, P], dt.bfloat16, tag="wh")
                            nc.sync.dma_start(wh[:], vhi[j, m])
                            wl = wp.tile([P, KO, P], dt.bfloat16, tag="wl")
                            nc.sync.dma_start(wl[:], vlo[j, m])
                        steps = []
                        # term-major: hi@hi consumes chunks in arrival order
                        for k in range(KO):
                            steps.append((wh, z_hi[j], k))
                        for k in range(KO):
                            steps.append((wh, z_lo[j], k))
                        for k in range(KO):
                            steps.append((wl, z_hi[j], k))
                    else:
                        if first:
                            w = w0
                        else:
                            w = wp.tile([P, KO, P], vt.dtype, tag="w")
                            nc.sync.dma_start(w[:], vt[j, m])
                        steps = [(w, z_tiles[j], k) for k in range(KO)]

                    pt = ps.tile([P, C], dt.float32, tag="psum")
                    n_mm = len(steps)
                    for i, (wt_t, zch, k) in enumerate(steps):
                        nc.tensor.matmul(
                            pt[:],
                            wt_t[:, k, :],
                            zch[k // KCH][:, k % KCH, :],
                            start=(i == 0),
                            stop=(i == n_mm - 1),
                        )
                    o = op.tile([P, C], dt.float32, tag="out")
                    # explicit DVE: nc.any routes this to ScalarE (9x slower)
                    nc.vector.tensor_copy(out=o[:], in_=pt[:])
                    nc.sync.dma_start(yt[j, m], o[:])

    nc.compile()
    return nc


_NC_CACHE: dict = {}


def _get_nc(strategy: str):
    if strategy not in _NC_CACHE:
        _NC_CACHE[strategy] = build_nc(strategy)
    return _NC_CACHE[strategy]


def _prepare_in_maps(logits, prior_logits, W, W_prior, expert_idx, strategy):
    """Route tokens and build the 8 per-core input maps.

    Returns (in_maps, routed_idx, overflow) where routed_idx[e] is the array
    of token indices assigned to expert e's capacity slots (in slot order)
    and overflow is the list of (token_idx, expert) that did not fit.
    """
    in_maps = []
    routed_idx = []
    overflow = []
    for e in range(E):
        idx = np.nonzero(expert_idx == e)[0]
        if len(idx) > C:
            overflow.extend((int(i), e) for i in idx[C:])
            idx = idx[:C]
        routed_idx.append(idx)

    for c in range(N_CORES):
        zt_c = np.zeros((2, P, KO, C), np.float32)
        vt_c = np.empty((2, MO, P, KO, P), np.float32)
        for j in range(EXPERTS_PER_CORE):
            e = EXPERTS_PER_CORE * c + j
            idx = routed_idx[e]
            n_e = len(idx)
            # Z = [X_e.T ; Xp_e.T]  -> [K, C] -> tiled [P, KO, C]
            Z = np.zeros((K, C), np.float32)
            Z[:IN_F, :n_e] = logits[idx].T
            Z[IN_F:, :n_e] = prior_logits[idx].T
            zt_c[j] = Z.reshape(KO, P, C).transpose(1, 0, 2)
            # V = [W[e].T ; Wp[e].T] -> [K, OUT_F] -> tiled [MO, P, KO, P]
            V = np.concatenate([W[e].T, W_prior[e].T], axis=0)
            vt_c[j] = V.reshape(KO, P, MO, P).transpose(2, 1, 0, 3)

        if strategy == "bf16x3":
            zhi, zlo = _split_bf16(zt_c)
            vhi, vlo = _split_bf16(vt_c)
            in_maps.append({"zhi": zhi, "zlo": zlo, "vhi": vhi, "vlo": vlo})
        elif strategy == "fp32r":
            in_maps.append({"zt": _round_fp32r(zt_c), "vt": _round_fp32r(vt_c)})
        else:
            in_maps.append({"zt": zt_c, "vt": vt_c})
    return in_maps, routed_idx, overflow


def _gather_output(results, routed_idx, overflow, logits, prior_logits, W, b,
                   W_prior, b_prior, expert_idx):
    out = np.empty((B, OUT_F), np.float32)
    for c in range(N_CORES):
        yt = results[c]["yt"]  # [2, MO, P, C]
        for j in range(EXPERTS_PER_CORE):
            e = EXPERTS_PER_CORE * c + j
            idx = routed_idx[e]
            if len(idx) == 0:
                continue
            y = yt[j].reshape(OUT_F, C)  # [out, C]
            out[idx] = y[:, : len(idx)].T
    # bias (zero in this problem, added for faithfulness)
    bias = b + b_prior  # [E, OUT_F]
    if np.any(bias):
        out += bias[expert_idx]
    for i, e in overflow:
        out[i] = (
            W[e] @ logits[i]
            + b[e]
            + W_prior[e] @ prior_logits[i]
            + b_prior[e]
        )
    return out[:, None, :]


def run(inputs: dict, strategy: str = STRATEGY, trace: bool = False):
    """Run the kernel; returns (output, BassKernelResults)."""
    logits = np.asarray(inputs["logits"], np.float32)
    prior_logits = np.asarray(inputs["prior_logits"], np.float32)
    W = np.asarray(inputs["W"], np.float32)
    b = np.asarray(inputs["b"], np.float32)
    W_prior = np.asarray(inputs["W_prior"], np.float32)
    b_prior = np.asarray(inputs["b_prior"], np.float32)
    expert_idx = np.asarray(inputs["expert_idx"])

    nc = _get_nc(strategy)
    in_maps, routed_idx, overflow = _prepare_in_maps(
        logits, prior_logits, W, W_prior, expert_idx, strategy
    )
    br = run_bass_kernel_spmd(nc, in_maps, list(range(N_CORES)), trace=trace)
    out = _gather_output(
        br.results, routed_idx, overflow, logits, prior_logits, W, b, W_prior,
        b_prior, expert_idx,
    )
    return out, br


def kernel(**inputs) -> np.ndarray:
    out, _ = run(inputs, STRATEGY)
    return out


# revision 10
# speedup vs baseline: 1.3932x; 1.3932x over previous
"""Trainium2 Bass kernel for nn_EnsembleLayer (MoE one-hot routing).

Reference math (per token i, expert e = expert_idx[i]):
    out[i] = W[e] @ logits[i] + b[e] + W_prior[e] @ prior_logits[i] + b_prior[e]

Strategy:
  * Host-side routing: sort tokens by expert (the reference wastefully
    computes all 16 experts per token; we compute only the routed one).
  * Expert-parallel over 8 NeuronCores: core c owns experts (2c, 2c+1),
    each padded to a static capacity of C tokens.
  * Per expert slot the device computes  V.T @ Z  where
        Z = [X_e.T ; Xp_e.T]          (K=2048, C tokens)
        V = [W[e].T ; W_prior[e].T]   (K=2048, 1024 out)
    i.e. posterior and prior fused into one K=2048 contraction, both
    accumulating in the same PSUM tile.
  * Bias (zero in this problem, but handled anyway) is added on host.
  * Tokens overflowing the static capacity (cannot happen for the fixed
    seed's counts, max 295 < C) fall back to exact numpy on host.

Precision strategy for the matmuls (PE TensorEngine):
  * "fp32r" : single-pass FP32R (1s+8e+11m, 1 cyc/row at N>=256). ~1.5e-4
              max scale-relative error.
  * "bf16x3": bf16 hi/lo split, 3 accumulating passes
              (hi@hi + hi@lo + lo@hi). ~5e-6 max scale-relative error.
  * "fp32"  : native fp32 matmul (4 cyc/row). ~2e-7.
"""

import sys

sys.path.insert(0, "/opt/trn_rl_repo")

import ml_dtypes
import numpy as np

import concourse.mybir as mybir
import concourse.tile as tile
from concourse import bacc
from concourse.bass_utils import run_bass_kernel_spmd

dt = mybir.dt

# ---- problem constants (hardcoded per contract) ----
B = 4096
IN_F = 1024
OUT_F = 1024
E = 16
N_CORES = 8
EXPERTS_PER_CORE = E // N_CORES  # 2
P = 128
K = 2 * IN_F  # 2048: posterior + prior fused contraction
KO = K // P  # 16 k-tiles
MO = OUT_F // P  # 8 m-tiles
C = 320  # static token capacity per expert (seed-0 max count is 295)

STRATEGY = "bf16x3"  # one of: "fp32r", "bf16x3", "fp32"


def _round_fp32r(x: np.ndarray) -> np.ndarray:
    """Round fp32 to the FP32R grid (1s+8e+11m, RN-even)."""
    u = np.ascontiguousarray(x).view(np.uint32)
    r = (u + np.uint32(0x7FF) + ((u >> np.uint32(12)) & np.uint32(1))) & np.uint32(
        0xFFFFF000
    )
    return r.view(np.float32)


def _split_bf16(x: np.ndarray):
    hi = x.astype(ml_dtypes.bfloat16)
    lo = (x - hi.astype(np.float32)).astype(ml_dtypes.bfloat16)
    return hi, lo


def build_nc(strategy: str = STRATEGY):
    """Build the SPMD per-core Bass program.

    Inputs per core (leading dim j = expert slot 0/1):
      zt  [2, P, KO, C]      tokens, k-tiled transposed   (fp32r / fp32)
      vt  [2, MO, P, KO, P]  weights, k&m-tiled           (fp32r / fp32)
      (bf16x3: zhi/zlo and vhi/vlo in bf16 instead)
    Output:
      yt  [2, MO, P, C] fp32   yt[j,m,mi,n] = out-feature m*P+mi of token n
    """
    nc = bacc.Bacc("TRN2", target_bir_lowering=False, debug=False)

    if strategy == "bf16x3":
        zhi = nc.dram_tensor("zhi", [2, P, KO, C], dt.bfloat16, kind="ExternalInput").ap()
        zlo = nc.dram_tensor("zlo", [2, P, KO, C], dt.bfloat16, kind="ExternalInput").ap()
        vhi = nc.dram_tensor("vhi", [2, MO, P, KO, P], dt.bfloat16, kind="ExternalInput").ap()
        vlo = nc.dram_tensor("vlo", [2, MO, P, KO, P], dt.bfloat16, kind="ExternalInput").ap()
    else:
        mm_dt = dt.float32r if strategy == "fp32r" else dt.float32
        zt = nc.dram_tensor("zt", [2, P, KO, C], mm_dt, kind="ExternalInput").ap()
        vt = nc.dram_tensor("vt", [2, MO, P, KO, P], mm_dt, kind="ExternalInput").ap()
    yt = nc.dram_tensor("yt", [2, MO, P, C], dt.float32, kind="ExternalOutput").ap()

    with tile.TileContext(nc) as tc:
        with (
            tc.tile_pool(name="zp", bufs=1) as zp,
            tc.tile_pool(name="wp", bufs=4) as wp,
            tc.tile_pool(name="ps", bufs=8, space="PSUM") as ps,
            tc.tile_pool(name="op", bufs=4) as op,
        ):
            # Token tiles are chunked along k (KCH k-tiles per chunk) so the
            # first matmuls only wait on the first small DMA instead of the
            # full 5MB token load — PE starts ~17us earlier.
            KCH = 2
            NCH = KO // KCH

            def load_chunked(dram_ap, dtype, tag, nch, sub):
                """Allocate nch tiles [P, KO//nch, C-like] and DMA each."""
                tiles = []
                for ci in range(nch):
                    t = zp.tile([P, KO // nch, dram_ap.shape[-1]], dtype,
                                tag=f"{tag}c{ci}")
                    tiles.append(t)
                # DMAs issued by caller in the order it wants
                return tiles

            def issue(tiles, dram_ap, nch):
                kw = KO // nch
                for ci, t in enumerate(tiles):
                    nc.sync.dma_start(t[:], dram_ap[:, ci * kw : (ci + 1) * kw])

            # Issue order shapes delivery: group-0 weights + j=0 tokens come
            # first; j=1 tokens (not needed until group 8) are drip-fed from
            # inside the group loop so groups 1..7's weights aren't queued
            # behind them.
            kw = KO // NCH

            def issue_chunk(tiles, dram_ap, ci):
                t = tiles[ci]
                nc.sync.dma_start(t[:], dram_ap[:, ci * kw : (ci + 1) * kw])

            KH = KO // 2

            def half_tiles(dram_ap, dtype, tag):
                """Group-0 weights as two half-k tiles: the first matmuls wait
                only on the first half's DMA (deps are tile-granular)."""
                a = wp.tile([P, KH, P], dtype, tag=tag)
                b = wp.tile([P, KH, P], dtype, tag=tag)
                return a, b

            pending = []
            if strategy == "bf16x3":
                z_hi = [load_chunked(zhi[j], dt.bfloat16, f"zhi{j}", NCH, C) for j in range(2)]
                z_lo = [load_chunked(zlo[j], dt.bfloat16, f"zlo{j}", NCH, C) for j in range(2)]
                wh0a, wh0b = half_tiles(vhi[0, 0], dt.bfloat16, "wh")
                wl0a, wl0b = half_tiles(vlo[0, 0], dt.bfloat16, "wl")
                nc.sync.dma_start(wh0a[:], vhi[0, 0, :, :KH])
                issue_chunk(z_hi[0], zhi[0], 0)
                nc.sync.dma_start(wh0b[:], vhi[0, 0, :, KH:])
                for ci in range(1, NCH):
                    issue_chunk(z_hi[0], zhi[0], ci)
                issue_chunk(z_lo[0], zlo[0], 0)
                nc.sync.dma_start(wl0a[:], vlo[0, 0, :, :KH])
                for ci in range(1, NCH):
                    issue_chunk(z_lo[0], zlo[0], ci)
                nc.sync.dma_start(wl0b[:], vlo[0, 0, :, KH:])
                for ci in range(NCH):
                    pending.append((z_hi[1], zhi[1], ci))
                    pending.append((z_lo[1], zlo[1], ci))
            else:
                z_tiles = [load_chunked(zt[j], zt.dtype, f"z{j}", NCH, C) for j in range(2)]
                w0a, w0b = half_tiles(vt[0, 0], vt.dtype, "w")
                nc.sync.dma_start(w0a[:], vt[0, 0, :, :KH])
                issue_chunk(z_tiles[0], zt[0], 0)
                nc.sync.dma_start(w0b[:], vt[0, 0, :, KH:])
                for ci in range(1, NCH):
                    issue_chunk(z_tiles[0], zt[0], ci)
                for ci in range(NCH):
                    pending.append((z_tiles[1], zt[1], ci))

            for j in range(2):
                for m in range(MO):
                    first = j == 0 and m == 0
                    # drip-feed the j=1 token chunks during early j=0 groups
                    if j == 0 and 3 <= m <= 6 and pending:
                        for _ in range(4):
                            if pending:
                                tiles_, ap_, ci_ = pending.pop(0)
                                issue_chunk(tiles_, ap_, ci_)
                    if strategy == "bf16x3":
                        if first:
                            wh_at = lambda k: (wh0a, k) if k < KH else (wh0b, k - KH)
                            wl_at = lambda k: (wl0a, k) if k < KH else (wl0b, k - KH)
                        else:
                            wh = wp.tile([P, KO, P], dt.bfloat16, tag="wh")
                            nc.sync.dma_start(wh[:], vhi[j, m])
                            wl = wp.tile([P, KO, P], dt.bfloat16, tag="wl")
                            nc.sync.dma_start(wl[:], vlo[j, m])
                            wh_at = lambda k: (wh, k)
                            wl_at = lambda k: (wl, k)
                        steps = []
                        # term-major: hi@hi consumes chunks in arrival order
                        for k in range(KO):
                            steps.append((*wh_at(k), z_hi[j], k))
                        for k in range(KO):
                            steps.append((*wh_at(k), z_lo[j], k))
                        for k in range(KO):
                            steps.append((*wl_at(k), z_hi[j], k))
                    else:
                        if first:
                            w_at = lambda k: (w0a, k) if k < KH else (w0b, k - KH)
                        else:
                            w = wp.tile([P, KO, P], vt.dtype, tag="w")
                            nc.sync.dma_start(w[:], vt[j, m])
                            w_at = lambda k: (w, k)
                        steps = [(*w_at(k), z_tiles[j], k) for k in range(KO)]

                    pt = ps.tile([P, C], dt.float32, tag="psum")
                    n_mm = len(steps)
                    for i, (wt_t, wk, zch, k) in enumerate(steps):
                        nc.tensor.matmul(
                            pt[:],
                            wt_t[:, wk, :],
                            zch[k // KCH][:, k % KCH, :],
                            start=(i == 0),
                            stop=(i == n_mm - 1),
                        )
                    o = op.tile([P, C], dt.float32, tag="out")
                    # explicit DVE: nc.any routes this to ScalarE (9x slower)
                    nc.vector.tensor_copy(out=o[:], in_=pt[:])
                    nc.sync.dma_start(yt[j, m], o[:])

    nc.compile()
    return nc


_NC_CACHE: dict = {}


def _get_nc(strategy: str):
    if strategy not in _NC_CACHE:
        _NC_CACHE[strategy] = build_nc(strategy)
    return _NC_CACHE[strategy]


def _prepare_in_maps(logits, prior_logits, W, W_prior, expert_idx, strategy):
    """Route tokens and build the 8 per-core input maps.

    Returns (in_maps, routed_idx, overflow) where routed_idx[e] is the array
    of token indices assigned to expert e's capacity slots (in slot order)
    and overflow is the list of (token_idx, expert) that did not fit.
    """
    in_maps = []
    routed_idx = []
    overflow = []
    for e in range(E):
        idx = np.nonzero(expert_idx == e)[0]
        if len(idx) > C:
            overflow.extend((int(i), e) for i in idx[C:])
            idx = idx[:C]
        routed_idx.append(idx)

    for c in range(N_CORES):
        zt_c = np.zeros((2, P, KO, C), np.float32)
        vt_c = np.empty((2, MO, P, KO, P), np.float32)
        for j in range(EXPERTS_PER_CORE):
            e = EXPERTS_PER_CORE * c + j
            idx = routed_idx[e]
            n_e = len(idx)
            # Z = [X_e.T ; Xp_e.T]  -> [K, C] -> tiled [P, KO, C]
            Z = np.zeros((K, C), np.float32)
            Z[:IN_F, :n_e] = logits[idx].T
            Z[IN_F:, :n_e] = prior_logits[idx].T
            zt_c[j] = Z.reshape(KO, P, C).transpose(1, 0, 2)
            # V = [W[e].T ; Wp[e].T] -> [K, OUT_F] -> tiled [MO, P, KO, P]
            V = np.concatenate([W[e].T, W_prior[e].T], axis=0)
            vt_c[j] = V.reshape(KO, P, MO, P).transpose(2, 1, 0, 3)

        if strategy == "bf16x3":
            zhi, zlo = _split_bf16(zt_c)
            vhi, vlo = _split_bf16(vt_c)
            in_maps.append({"zhi": zhi, "zlo": zlo, "vhi": vhi, "vlo": vlo})
        elif strategy == "fp32r":
            in_maps.append({"zt": _round_fp32r(zt_c), "vt": _round_fp32r(vt_c)})
        else:
            in_maps.append({"zt": zt_c, "vt": vt_c})
    return in_maps, routed_idx, overflow


def _gather_output(results, routed_idx, overflow, logits, prior_logits, W, b,
                   W_prior, b_prior, expert_idx):
    out = np.empty((B, OUT_F), np.float32)
    for c in range(N_CORES):
        yt = results[c]["yt"]  # [2, MO, P, C]
        for j in range(EXPERTS_PER_CORE):
            e = EXPERTS_PER_CORE * c + j
            idx = routed_idx[e]
            if len(idx) == 0:
                continue
            y = yt[j].reshape(OUT_F, C)  # [out, C]
            out[idx] = y[:, : len(idx)].T
    # bias (zero in this problem, added for faithfulness)
    bias = b + b_prior  # [E, OUT_F]
    if np.any(bias):
        out += bias[expert_idx]
    for i, e in overflow:
        out[i] = (
            W[e] @ logits[i]
            + b[e]
            + W_prior[e] @ prior_logits[i]
            + b_prior[e]
        )
    return out[:, None, :]


def run(inputs: dict, strategy: str = STRATEGY, trace: bool = False):
    """Run the kernel; returns (output, BassKernelResults)."""
    logits = np.asarray(inputs["logits"], np.float32)
    prior_logits = np.asarray(inputs["prior_logits"], np.float32)
    W = np.asarray(inputs["W"], np.float32)
    b = np.asarray(inputs["b"], np.float32)
    W_prior = np.asarray(inputs["W_prior"], np.float32)
    b_prior = np.asarray(inputs["b_prior"], np.float32)
    expert_idx = np.asarray(inputs["expert_idx"])

    nc = _get_nc(strategy)
    in_maps, routed_idx, overflow = _prepare_in_maps(
        logits, prior_logits, W, W_prior, expert_idx, strategy
    )
    br = run_bass_kernel_spmd(nc, in_maps, list(range(N_CORES)), trace=trace)
    out = _gather_output(
        br.results, routed_idx, overflow, logits, prior_logits, W, b, W_prior,
        b_prior, expert_idx,
    )
    return out, br


def kernel(**inputs) -> np.ndarray:
    out, _ = run(inputs, STRATEGY)
    return out
